# revision 19
# baseline (speedup 1.0000x reference)
# DropConnect LSTM cell kernel for Trainium2 (Bass/Tile).
#
# Math (per reference):
#   x_d = x * (dp_u >= 0.1) / 0.9
#   h_d = h * (rec_dp_u >= 0.1) / 0.9
#   w   = kernel * (k_dp_u >= 0.05) / 0.95
#   rw  = recurrent_kernel * (rk_dp_u >= 0.05) / 0.95
#   z   = x_d @ w + h_d @ rw + bias          (split into gates i,f,c~,o)
#   c'  = sig(zf)*c + sig(zi)*tanh(zc)
#   h'  = sig(zo)*tanh(c')
#
# Sharding: 4 batch-quarters x 2 gate-halves across 8 cores. Core c handles
# batch rows [bq*2048, (bq+1)*2048) and u-columns [hf*512, (hf+1)*512) of all
# four gates, with bq, hf = divmod(c, 2). All slicing happens host-side in
# make_in_maps. The kernel is HBM-bound, so x/h/c/kernel/recurrent_kernel are
# also staged host-side as fp16 — they are consumed as fp16 matmul operands
# anyway, so the rounding point is identical and the device output is
# bit-identical to casting on-chip; the four dropout-uniform tensors stay
# f32 so every >= threshold compare is exact. Per-core working set: ~50MB
# (32MB of it the f32 uniforms), ~128us of DMA vs ~140us of PE.
#
# Kernel strategy (per core, 2048 batch rows x 2048 gate cols):
#  - All matmul operands fp16 (same PE rate as bf16, 4x less rounding error;
#    rel err ~1.3e-3 vs 9.9e-3 for bf16). Outputs written fp16 (adds ~5e-4
#    relative rounding, upcast to f32 on the host).
#  - Combined dropout scale S = 1/(0.9*0.95) applied once inside the gate
#    activations (out = f(S*psum)); bias pre-divided by S and injected into
#    PSUM with a K=1 matmul so psum = act@w + hact@rw + bias/S.
#  - Masked fp16 weights for the core's 2048 gate-cols stay SBUF-resident
#    (64KB/partition); activations are masked on DVE, transposed on PE
#    (PSUM), and cast to fp16 by the Act engine on copy-out.
#  - Work grouped by b-tile: 4 PSUM banks hold zi,zf,zc,zo for 128 rows x
#    512 u-cols; gate math (Act sigmoid/tanh + gpsimd muls) follows the 64
#    matmuls of the group.

from contextlib import ExitStack

import numpy as np

import concourse.bass as bass
import concourse.mybir as mybir
import concourse.tile as tile
from concourse import bacc
from concourse.bass_utils import run_bass_kernel_spmd
from concourse.masks import make_identity

N_CORES = 8
B, D, U = 8192, 1024, 1024
BSHARD, GSHARD = 4, 2
BC = B // BSHARD  # per-core batch rows (2048)
NW = 512  # u-cols per gate per core (one psum bank)
GC = 4 * NW  # 2048 gate cols per core
P = 128
KX = D // P  # 8 x-path contraction tiles
KH = U // P  # 8 h-path contraction tiles

DROPOUT = 0.1
KERNEL_DROPOUT = 0.05
S = 1.0 / ((1.0 - DROPOUT) * (1.0 - KERNEL_DROPOUT))

f32 = mybir.dt.float32
f16 = mybir.dt.float16
AF = mybir.ActivationFunctionType
OP = mybir.AluOpType


def build_nc(bc: int = BC, repeat: int = 1, loop: int = 1, diag: str = ""):
    """Build and compile the per-core Bass program for per-core batch bc.

    repeat > 1 re-emits the whole computation N times in one NEFF; loop > 1
    wraps the body in a hardware For_i loop — both only for timing runs.
    """
    nc = bacc.Bacc("TRN2", target_bir_lowering=False, debug=False)

    # x/h/kern/rkern are staged host-side as fp16 (they are consumed as fp16
    # matmul operands anyway, so the rounding point is identical); the
    # dropout uniforms stay f32 so the >= threshold compares are exact.
    x = nc.dram_tensor("x", [bc, D], f16, kind="ExternalInput").ap()
    h = nc.dram_tensor("h", [bc, U], f16, kind="ExternalInput").ap()
    c_in = nc.dram_tensor("c", [bc, NW], f16, kind="ExternalInput").ap()
    dp = nc.dram_tensor("dp_u", [bc, D], f32, kind="ExternalInput").ap()
    rdp = nc.dram_tensor("rec_dp_u", [bc, U], f32, kind="ExternalInput").ap()
    kw = nc.dram_tensor("kern", [D, GC], f16, kind="ExternalInput").ap()
    rkw = nc.dram_tensor("rkern", [U, GC], f16, kind="ExternalInput").ap()
    kdp = nc.dram_tensor("k_dp_u", [D, GC], f32, kind="ExternalInput").ap()
    rkdp = nc.dram_tensor("rk_dp_u", [U, GC], f32, kind="ExternalInput").ap()
    bias = nc.dram_tensor("bias", [GC], f32, kind="ExternalInput").ap()
    h_new = nc.dram_tensor("h_new", [bc, NW], f16, kind="ExternalOutput").ap()
    c_new = nc.dram_tensor("c_new", [bc, NW], f16, kind="ExternalOutput").ap()

    with tile.TileContext(nc) as tc, ExitStack() as ctx:
        const = ctx.enter_context(tc.tile_pool(name="const", bufs=1))
        astage = ctx.enter_context(tc.tile_pool(name="astage", bufs=3))
        amask = ctx.enter_context(tc.tile_pool(name="amask", bufs=2))
        atrans = ctx.enter_context(tc.tile_pool(name="atrans", bufs=1))
        wstage = ctx.enter_context(tc.tile_pool(name="wstage", bufs=2))
        wxpool = ctx.enter_context(tc.tile_pool(name="wx", bufs=1))
        gstage = ctx.enter_context(tc.tile_pool(name="gstage", bufs=6))
        cpool = ctx.enter_context(tc.tile_pool(name="cpool", bufs=3))
        bstage = ctx.enter_context(tc.tile_pool(name="bstage", bufs=1))
        psum = ctx.enter_context(tc.tile_pool(name="psum", bufs=6, space="PSUM"))
        psumt = ctx.enter_context(tc.tile_pool(name="psumt", bufs=2, space="PSUM"))

        ident = const.tile([P, P], f16)
        make_identity(nc, ident)
        ones1 = const.tile([1, P], f16)
        nc.vector.memset(ones1, 1.0 / S)

        def emit_all():
            for _rep in range(repeat):
                emit_body(
                    nc, tc, bc,
                    x, h, c_in, dp, rdp, kw, rkw, kdp, rkdp, h_new, c_new,
                    astage, amask, atrans, wstage, wxpool, gstage,
                    cpool, bstage, psum, psumt, ident, ones1, bias, diag,
                )

        if loop > 1:
            with tc.For_i(0, loop, 1):
                emit_all()
        else:
            emit_all()

    nc.compile()
    return nc


def emit_body(
    nc, tc, bc,
    x, h, c_in, dp, rdp, kw, rkw, kdp, rkdp, h_new, c_new,
    astage, amask, atrans, wstage, wxpool, gstage,
    cpool, bstage, psum, psumt, ident, ones1, bias, diag="",
):
    btl = bc // P

    # ---- bias: casting gpsimd DMA straight into fp16 [1, 4, NW] ----
    bias_all = bstage.tile([1, 4, NW], f16, tag="bs", name="bias_all")
    nc.gpsimd.dma_start(out=bias_all, in_=bias.unsqueeze(0).rearrange("p (g w) -> p g w", g=4))

    # ---- weight DMA + mask (DVE); result resident in SBUF ----
    wmx = [None] * KX
    wmh = [None] * KH

    def emit_w(kind, kk):
        wm_arr = wmx if kind == "x" else wmh
        if "fakew" in diag:
            if wmx[0] is None:
                wm = wxpool.tile([P, 4, NW], f16, tag="wfake", name="wfake")
                nc.vector.memset(wm, 0.01)
                wmx[0] = wm
            wm_arr[kk] = wmx[0]
            return
        src, usrc = (kw, kdp) if kind == "x" else (rkw, rkdp)
        r0 = kk * P
        wt = wstage.tile([P, 4, NW], f16, tag="wraw16")
        uw = wstage.tile([P, 4, NW], f32, tag="wraw")
        nc.sync.dma_start(out=wt, in_=src[r0 : r0 + P, :].rearrange("p (g w) -> p g w", g=4))
        nc.sync.dma_start(out=uw, in_=usrc[r0 : r0 + P, :].rearrange("p (g w) -> p g w", g=4))
        wm = wxpool.tile([P, 4, NW], f16, tag=f"w{kind}{kk}", name=f"w{kind}{kk}")
        nc.vector.scalar_tensor_tensor(
            wm, uw, KERNEL_DROPOUT, wt, op0=OP.is_ge, op1=OP.mult
        )
        wm_arr[kk] = wm

    # ---- Phase A job: mask acts, transpose on PE, cast on copy-out ----
    actTx = [None] * btl
    actTh = [None] * btl

    def emit_acts(b):
        rows = slice(b * P, (b + 1) * P)
        actTx[b] = atrans.tile([P, KX, P], f16, name=f"aTx{b}", tag=f"aT{b}x")
        actTh[b] = atrans.tile([P, KH, P], f16, name=f"aTh{b}", tag=f"aT{b}h")
        for src, usrc, dst, nk in ((x, dp, actTx[b], KX), (h, rdp, actTh[b], KH)):
            vt = astage.tile([P, D], f16, tag="araw16")
            ut = astage.tile([P, D], f32, tag="araw")
            nc.sync.dma_start(out=vt, in_=src[rows, :])
            nc.sync.dma_start(out=ut, in_=usrc[rows, :])
            vm = amask.tile([P, D], f16, tag="am")
            nc.vector.scalar_tensor_tensor(
                vm, ut, DROPOUT, vt, op0=OP.is_ge, op1=OP.mult
            )
            pt = psumt.tile([P, nk, P], f16, tag="zt", name=f"tp{b}_{nk}")
            for j in range(nk):
                nc.tensor.transpose(pt[:, j, :], vm[:, j * P : (j + 1) * P], ident)
            nc.scalar.copy(dst, pt)

    # ---- Group: 4 psum banks = zi,zf,zc,zo for one b-tile ----
    def emit_group(b):
        rows = slice(b * P, (b + 1) * P)
        ct = cpool.tile([P, NW], f16, tag="ct", name=f"ct{b}")
        nc.sync.dma_start(out=ct, in_=c_in[rows, :])
        z = [psum.tile([P, NW], f32, tag="z", name=f"z{b}_{g}") for g in range(4)]
        for g in range(4):
            nc.tensor.matmul(
                z[g], lhsT=ones1, rhs=bias_all[:, g, :], start=True, stop=False
            )
        xrep = 2 if "xtra" in diag else 1
        for _xr in range(xrep):
            for kk in range(KX):
                for g in range(4):
                    nc.tensor.matmul(
                        z[g], lhsT=actTx[b][:, kk, :], rhs=wmx[kk][:, g, :],
                        start=False, stop=False,
                    )
        hrep = 2 if "htra" in diag else 1
        for _hr in range(hrep):
            last_rep = _hr == hrep - 1
            for kk in range(KH):
                for g in range(4):
                    nc.tensor.matmul(
                        z[g], lhsT=actTh[b][:, kk, :], rhs=wmh[kk][:, g, :],
                        start=False, stop=(last_rep and kk == KH - 1),
                    )
        if "nogates" in diag:
            for g in range(4):
                sg = gstage.tile([P, NW], f32, tag="g", name=f"dg{b}_{g}")
                nc.scalar.copy(sg, z[g])
            nc.sync.dma_start(out=h_new[rows, :], in_=sg)
            return
        si = gstage.tile([P, NW], f32, tag="g", name=f"si{b}")
        tcc = gstage.tile([P, NW], f32, tag="g", name=f"tcc{b}")
        sf = gstage.tile([P, NW], f32, tag="g", name=f"sf{b}")
        so = gstage.tile([P, NW], f32, tag="g", name=f"so{b}")
        cn = gstage.tile([P, NW], f16, tag="g16", name=f"cn{b}")
        hn = gstage.tile([P, NW], f16, tag="g16", name=f"hn{b}")
        nc.scalar.activation(si, z[0], AF.Sigmoid, scale=S)
        nc.scalar.activation(tcc, z[2], AF.Tanh, scale=S)
        nc.scalar.activation(sf, z[1], AF.Sigmoid, scale=S)
        nc.scalar.activation(so, z[3], AF.Sigmoid, scale=S)
        nc.gpsimd.tensor_tensor(si, si, tcc, OP.mult)      # i*tanh(zc)
        nc.gpsimd.tensor_tensor(sf, sf, ct, OP.mult)       # f*c
        nc.gpsimd.tensor_tensor(cn, si, sf, OP.add)        # c'
        nc.sync.dma_start(out=c_new[rows, :], in_=cn)
        nc.scalar.activation(tcc, cn, AF.Tanh)             # tanh(c')
        nc.gpsimd.tensor_tensor(hn, so, tcc, OP.mult)      # h'
        nc.sync.dma_start(out=h_new[rows, :], in_=hn)

    # ---- emission order: interleave weight and act jobs so the DMA
    # pipeline stays saturated while the PE transposes early b-tiles;
    # groups follow and start as soon as their deps land.
    for i in range(max(KX, btl // 2)):
        if i < KX:
            emit_w("x", i)
        if 2 * i < btl:
            emit_acts(2 * i)
        if 2 * i + 1 < btl:
            emit_acts(2 * i + 1)
        if i < KH:
            emit_w("h", i)
    for b in range(btl):
        emit_group(b)


_NC_CACHE: dict[tuple, object] = {}


def get_nc(bc: int = BC, repeat: int = 1, loop: int = 1, diag: str = ""):
    key = (bc, repeat, loop, diag)
    if key not in _NC_CACHE:
        _NC_CACHE[key] = build_nc(bc, repeat, loop, diag)
    return _NC_CACHE[key]


def make_in_maps(x, h, c, kernel, recurrent_kernel, bias, dp_u, rec_dp_u, k_dp_u, rk_dp_u):
    def f(a):
        return np.ascontiguousarray(np.asarray(a, dtype=np.float32))

    def f16c(a):
        return np.ascontiguousarray(np.asarray(a, dtype=np.float32).astype(np.float16))

    x, h, c = f16c(x), f16c(h), f16c(c)
    dp_u, rec_dp_u = f(dp_u), f(rec_dp_u)
    kernel, recurrent_kernel, bias = f16c(kernel), f16c(recurrent_kernel), f(bias)
    k_dp_u, rk_dp_u = f(k_dp_u), f(rk_dp_u)

    # gate-half column slices: [D, 4*U] -> [D, 4, 2, NW] -> pick hf
    def gslice(w, hf):
        return np.ascontiguousarray(
            w.reshape(w.shape[0], 4, GSHARD, NW)[:, :, hf, :].reshape(w.shape[0], GC)
        )

    wh = [gslice(kernel, hf) for hf in range(GSHARD)]
    rwh = [gslice(recurrent_kernel, hf) for hf in range(GSHARD)]
    kdph = [gslice(k_dp_u, hf) for hf in range(GSHARD)]
    rkdph = [gslice(rk_dp_u, hf) for hf in range(GSHARD)]
    bh = [
        np.ascontiguousarray(bias.reshape(4, GSHARD, NW)[:, hf, :].reshape(GC))
        for hf in range(GSHARD)
    ]

    in_maps = []
    for ci in range(N_CORES):
        bq, hf = divmod(ci, GSHARD)
        bsl = slice(bq * BC, (bq + 1) * BC)
        usl = slice(hf * NW, (hf + 1) * NW)
        in_maps.append(
            {
                "x": np.ascontiguousarray(x[bsl]),
                "h": np.ascontiguousarray(h[bsl]),
                "c": np.ascontiguousarray(c[bsl, usl]),
                "dp_u": np.ascontiguousarray(dp_u[bsl]),
                "rec_dp_u": np.ascontiguousarray(rec_dp_u[bsl]),
                "kern": wh[hf],
                "rkern": rwh[hf],
                "k_dp_u": kdph[hf],
                "rk_dp_u": rkdph[hf],
                "bias": bh[hf],
            }
        )
    return in_maps


def assemble(res_list):
    h_new = np.empty((B, U), np.float32)
    c_new = np.empty((B, U), np.float32)
    for ci in range(N_CORES):
        bq, hf = divmod(ci, GSHARD)
        bsl = slice(bq * BC, (bq + 1) * BC)
        usl = slice(hf * NW, (hf + 1) * NW)
        h_new[bsl, usl] = np.float32(res_list[ci]["h_new"])
        c_new[bsl, usl] = np.float32(res_list[ci]["c_new"])
    return h_new, c_new


def kernel(x, h, c, kernel, recurrent_kernel, bias, dp_u, rec_dp_u, k_dp_u, rk_dp_u):
    nc = get_nc()
    in_maps = make_in_maps(
        x, h, c, kernel, recurrent_kernel, bias, dp_u, rec_dp_u, k_dp_u, rk_dp_u
    )
    res = run_bass_kernel_spmd(nc, in_maps, core_ids=list(range(N_CORES)))
    return assemble([res.results[ci] for ci in range(N_CORES)])
